# revision 18
# baseline (speedup 1.0000x reference)
"""Bass/Tile Trainium2 kernel for nn_CausalSelfAttention (B=4, T=2048, C=2048,
H=16 Q-heads, 4 KV-heads, RoPE, causal, fp32) distributed over 8 NeuronCores.

Sharding: tensor-parallel by head. Core c owns Q-heads {2c, 2c+1} and KV-head
c//2 (whole GQA groups). After attention on batch b, a per-batch AllToAll
redistributes the per-head outputs so every core computes the c_proj for a
256-token slice of each batch against the full Wo.

v3 design notes:
  - All stored tensors bf16 (fp32 PSUM accumulation). Besides halving DMA
    and enabling FWL, bf16 lifts the power throttle that pins fp32r matmul
    streams at k=13/16 (~1.95GHz): measured bf16 runs reach k=8/8 (2.4GHz).
  - q/k/v live in per-batch SBUF tiles (no DRAM round-trip), 2-batch
    pipeline via pool rotation.
  - exp runs on pairs of key chunks ([128,2,512], 573ns/chunk) so the ACT
    engine keeps pace with the PE's ~500ns/chunk score+pv stream.
  - Softmax denominator: bf16 DVE running sum of exp chunks, then 4 tiny
    matmuls put the per-query sums on 128 partitions ([128,4]), making the
    reciprocal a ~200ns DVE op. The 1/dn broadcast is a 2-DMA round-trip
    deferred two tiles so no engine FIFO blocks on its latency.
  - Emission order per batch: proj(b) | attn(b) | cproj(b-1) | a2a(b), so
    collectives always have a full batch of compute to hide behind and the
    input DMA queue (sync) never has a collective-dependent load at head.
  - Input DMAs (x, weights, a2a-out reads) on the sync queue; output DMAs
    (a2a-in, y, reciprocal round-trip) + collectives on the gpsimd queue.
  - PSUM: mm[2x2KB] rotation (proj/cproj/transposes/dnt) + sp2[2x4KB]
    (score pairs) + op[2x2KB] (PV accumulators) = exactly 8 banks.
"""

import numpy as np

B, T, C = 4, 2048, 2048
H, KV = 16, 4
D = C // H  # 128
BT = B * T  # 8192
N_CORES = 8
HPC = H // N_CORES  # q heads per core = 2
TPC = T // N_CORES  # tokens per core per batch for c_proj = 256
ROPE_BASE = 10000.0
NEG = -1.0e30

NTB = T // 512  # 4 projection/attention t-tiles per batch
NCH = T // 128  # 16 key chunks per batch

TRACE = False
LAST_EXEC_NS = None

_BUILT = None


def _build_program():
    import concourse.mybir as mybir
    import concourse.tile as tile
    from concourse import bacc
    from concourse.bass import ts

    f32 = mybir.dt.float32
    bf16 = mybir.dt.bfloat16
    Alu = mybir.AluOpType
    Act = mybir.ActivationFunctionType

    nc = bacc.Bacc("TRN2", target_bir_lowering=False, debug=False,
                   num_devices=N_CORES)

    # ---- I/O (all bf16 except the fp32 output) ----
    xT = nc.dram_tensor("xT", [C, BT], bf16, kind="ExternalInput")
    # packed [wq(2 heads) | wk | wv] -> [C, 512]
    wqkv = nc.dram_tensor("wqkv", [C, 4 * D], bf16, kind="ExternalInput")
    wo = nc.dram_tensor("wo", [C, C], bf16, kind="ExternalInput")
    # packed rope tables [4, D, T]: cosq, sinq, cosk, sink (q tables carry
    # the 1/sqrt(D) scale)
    rope = nc.dram_tensor("rope", [4, D, T], bf16, kind="ExternalInput")
    perm = nc.dram_tensor("perm", [D, D], bf16, kind="ExternalInput")
    cmask = nc.dram_tensor("cmask", [128, 4, 512], bf16, kind="ExternalInput")
    ones_col = nc.dram_tensor("ones_col", [128, 1], bf16, kind="ExternalInput")
    ident = nc.dram_tensor("ident", [128, 128], f32, kind="ExternalInput")
    identb = nc.dram_tensor("identb", [128, 128], bf16, kind="ExternalInput")
    y = nc.dram_tensor("y", [B, TPC, C], f32, kind="ExternalOutput")

    with tile.TileContext(nc) as tc:
        with (
            tc.tile_pool(name="const", bufs=1) as cp,
            tc.tile_pool(name="qkv", bufs=2) as kvp,
            tc.tile_pool(name="x", bufs=2) as xp,
            tc.tile_pool(name="work", bufs=2) as wp,
            tc.tile_pool(name="dram", bufs=1, space="DRAM") as dp,
            tc.tile_pool(name="rcp", bufs=4, space="DRAM") as rcp,
            tc.tile_pool(name="psum", bufs=2, space="PSUM") as pp_,
        ):
            xT_r = xT.ap().rearrange("(ko p) t -> p ko t", p=128)

            # ---- startup: first x tile, packed weights, rope tables ----
            xts = {}

            def load_xt(gt):  # gt = global tile index 0..15
                if gt < B * NTB and gt not in xts:
                    xt = xp.tile([128, 16, 512], bf16, tag="xt", name="xt")
                    nc.sync.dma_start(xt[:], xT_r[:, :, ts(gt, 512)])
                    xts[gt] = xt

            load_xt(0)
            wqkv_r = wqkv.ap().rearrange("(ko p) m -> p ko m", p=128)
            wqkv_sb = cp.tile([128, 16, 4 * D], bf16, name="wqkv_sb")
            nc.sync.dma_start(wqkv_sb[:], wqkv_r)
            rope_sb = cp.tile([D, 4, T], bf16, name="rope_sb")
            nc.sync.dma_start(rope_sb[:], rope.ap().rearrange("f d t -> d f t"))
            load_xt(1)
            perm_sb = cp.tile([D, D], bf16)
            nc.sync.dma_start(perm_sb[:], perm.ap())
            cmask_sb = cp.tile([128, 4, 512], bf16)
            nc.sync.dma_start(cmask_sb[:], cmask.ap())
            onec_sb = cp.tile([128, 1], bf16)
            nc.sync.dma_start(onec_sb[:], ones_col.ap())
            ident_sb = cp.tile([128, 128], f32)
            nc.sync.dma_start(ident_sb[:], ident.ap())
            identb_sb = cp.tile([128, 128], bf16)
            nc.sync.dma_start(identb_sb[:], identb.ap())

            # full Wo resident in SBUF (bf16, 64KB/partition), loaded in
            # quarters during batch 0 (first needed at cproj(0))
            wo_r = wo.ap().rearrange("(ko p) n -> p ko n", p=128)
            wo_sb = cp.tile([128, 16, C], bf16, name="wo_sb")

            def load_wo():
                for q in range(4):
                    nc.sync.dma_start(wo_sb[:, ts(q, 4), :],
                                      wo_r[:, ts(q, 4), :])

            # DRAM collective buffers, one pair per batch
            a2a_in = [dp.tile([N_CORES, HPC, D, TPC], bf16, name=f"a2a_in{b}")
                      for b in range(B)]
            a2a_out = [dp.tile([N_CORES, HPC, D, TPC], bf16, name=f"a2a_out{b}")
                       for b in range(B)]

            # deferred tail queues: stage 1 (reciprocal + broadcast DMA
            # round-trip) runs one attention tile late; stage 2 (normalize
            # multiply + a2a staging) two tiles late, once the broadcast has
            # landed, so no engine FIFO head-blocks on DMA latency.
            pend1, pend2 = [], []

            def flush_pending():
                while pend2:
                    pend2.pop(0)()
                while pend1:
                    pend2.append(pend1.pop(0)())
                while pend2:
                    pend2.pop(0)()

            def step_pending():
                while pend2:
                    pend2.pop(0)()
                while pend1:
                    pend2.append(pend1.pop(0)())

            # ================= phase 1: projections + RoPE (batch b) ========
            def proj_batch(b, qb, kb, vb):
                for tt in range(NTB):
                    gt = b * NTB + tt
                    xt = xts.pop(gt)
                    pos = tt * 512

                    cos_t = [rope_sb[:, 0, pos:pos + 512],
                             rope_sb[:, 0, pos:pos + 512],
                             rope_sb[:, 2, pos:pos + 512]]
                    sin_t = [rope_sb[:, 1, pos:pos + 512],
                             rope_sb[:, 1, pos:pos + 512],
                             rope_sb[:, 3, pos:pos + 512]]

                    pps, evs, t1s = [], [], []
                    for gi in range(4):
                        pqp = pp_.tile([128, 512], f32, tag="mm", bufs=2)
                        for k in range(16):
                            nc.tensor.matmul(pqp[:],
                                             wqkv_sb[:, k, ts(gi, 128)],
                                             xt[:, k, :],
                                             start=(k == 0), stop=(k == 15))
                        if gi < 3:
                            ev = wp.tile([128, 512], bf16, tag="ev", bufs=3)
                            nc.scalar.copy(ev[:], pqp[:])
                            # t1 emitted now so the "mm" slot frees early
                            t1 = wp.tile([128, 512], bf16, tag="t1", bufs=3)
                            nc.vector.tensor_tensor(t1[:], pqp[:], cos_t[gi],
                                                    op=Alu.mult)
                            t1s.append(t1)
                        else:
                            ev = wp.tile([128, 512], f32, tag="ev3", bufs=2)
                            nc.scalar.copy(ev[:], pqp[:])
                        evs.append(ev)

                    # rotate-half perm matmuls (t2 right behind each, so the
                    # mm-slot reader is already queued when the slot recycles)
                    t2s = []
                    for gi in range(3):
                        rp = pp_.tile([128, 512], f32, tag="mm", bufs=2)
                        nc.tensor.matmul(rp[:], perm_sb[:], evs[gi][:],
                                         start=True, stop=True)
                        t2 = wp.tile([128, 512], bf16, tag="t2", bufs=3)
                        nc.vector.tensor_tensor(t2[:], rp[:], sin_t[gi],
                                                op=Alu.mult)
                        t2s.append(t2)
                    # V transposes (fp32 to share the mm tag)
                    tp = pp_.tile([128, 512], f32, tag="mm", bufs=2)
                    for i in range(4):
                        nc.tensor.transpose(tp[:, ts(i, 128)],
                                            evs[3][:, ts(i, 128)], ident_sb[:])

                    load_xt(gt + 2)

                    # rope combine -> SBUF q/k tiles (bf16)
                    dsts = [qb[:, 0, pos:pos + 512], qb[:, 1, pos:pos + 512],
                            kb[:, pos:pos + 512]]
                    for gi in range(3):
                        nc.vector.tensor_tensor(dsts[gi], t1s[gi][:],
                                                t2s[gi][:], op=Alu.add)
                    for i in range(4):
                        nc.scalar.copy(vb[:, 4 * tt + i, :], tp[:, ts(i, 128)])

            # ================= phase 2: attention (batch b) =================
            def attn_batch(b, qb, kb, vb):
                for h in range(HPC):
                    for tt in range(NTB):
                        step_pending()
                        nch = 4 * (tt + 1)
                        npr = nch // 2
                        qt = qb[:, h, ts(tt, 512)]
                        op = pp_.tile([D, 512], f32, tag="op", bufs=2)
                        pts = []
                        acc = None

                        def emit_scores(j):
                            sp = pp_.tile([128, 2, 512], f32, tag="sp2",
                                          bufs=2)
                            for hf in range(2):
                                si = 2 * j + hf
                                diag = si >= 4 * tt
                                nc.tensor.matmul(sp[:, hf, :],
                                                 kb[:, ts(si, 128)], qt,
                                                 start=True, stop=not diag)
                                if diag:
                                    nc.tensor.matmul(
                                        sp[:, hf, :], identb_sb[:],
                                        cmask_sb[:, si - 4 * tt, :],
                                        start=False, stop=True)
                            pt = wp.tile([128, 2, 512], bf16, tag="pt",
                                         bufs=4)
                            nc.scalar.activation(pt[:], sp[:], Act.Exp)
                            pts.append(pt)

                        def emit_pv(j):
                            nonlocal acc
                            pt = pts[j]
                            for hf in range(2):
                                si = 2 * j + hf
                                nc.tensor.matmul(op[:], vb[:, si, :],
                                                 pt[:, hf, :],
                                                 start=(si == 0),
                                                 stop=(si == nch - 1))
                            if j == 0:
                                a = wp.tile([128, 512], bf16, tag="acc",
                                            bufs=2)
                                nc.vector.tensor_tensor(a[:], pt[:, 0, :],
                                                        pt[:, 1, :],
                                                        op=Alu.add)
                            else:
                                a = wp.tile([128, 512], bf16, tag="acc",
                                            bufs=2)
                                nc.vector.tensor_tensor(a[:], acc[:],
                                                        pt[:, 0, :],
                                                        op=Alu.add)
                                nc.vector.tensor_tensor(a[:], a[:],
                                                        pt[:, 1, :],
                                                        op=Alu.add)
                            acc = a

                        emit_scores(0)
                        for j in range(1, npr):
                            emit_scores(j)
                            emit_pv(j - 1)
                        emit_pv(npr - 1)

                        # denominator on partitions:
                        # dnt[p, j] = sum_s acc[s, 128j + p]
                        dnt = pp_.tile([128, 512], f32, tag="mm", bufs=2)
                        for j in range(4):
                            nc.tensor.matmul(dnt[:, j:j + 1],
                                             acc[:, ts(j, 128)], onec_sb[:],
                                             start=True, stop=True)

                        def tail1(b=b, h=h, tt=tt, op=op, dnt=dnt):
                            rct = wp.tile([128, 4], f32, tag="rct", bufs=2)
                            nc.vector.reciprocal_approx_fast(rct[:],
                                                             dnt[:, 0:4])
                            rcd = rcp.tile([512], f32, name="rcd")
                            nc.gpsimd.dma_start(
                                rcd.rearrange("(j p) -> p j", p=128), rct[:])
                            bcs = wp.tile([128, 512], f32, tag="bcs", bufs=2)
                            nc.gpsimd.dma_start(
                                bcs[:], rcd.rearrange("(a b) -> a b", a=1)
                                .to_broadcast((128, 512)))

                            def tail2(b=b, h=h, tt=tt, op=op, bcs=bcs):
                                osb = wp.tile([D, 512], bf16, tag="osb",
                                              bufs=2)
                                nc.vector.tensor_tensor(osb[:], op[:], bcs[:],
                                                        op=Alu.mult)
                                # two 256-token shards of the a2a input
                                for half in range(2):
                                    j = 2 * tt + half
                                    nc.gpsimd.dma_start(
                                        a2a_in[b][j, h, :, :],
                                        osb[:, ts(half, 256)])

                            return tail2

                        pend1.append(tail1)

            def emit_a2a(b):
                flush_pending()
                nc.gpsimd.collective_compute(
                    "AllToAll", mybir.AluOpType.bypass,
                    replica_groups=[list(range(N_CORES))],
                    ins=[a2a_in[b].opt()], outs=[a2a_out[b].opt()])

            # ================= phase 3: c_proj (batch b) ====================
            def cproj_batch(b):
                a2a_r = a2a_out[b].rearrange("i h d t -> d (i h) t")
                for tc_ in range(TPC // 128):
                    ot = wp.tile([128, 16, 128], bf16, tag="ot", bufs=2)
                    nc.sync.dma_start(ot[:], a2a_r[:, :, ts(tc_, 128)])
                    for on in range(4):
                        yp = pp_.tile([128, 512], f32, tag="mm", bufs=2)
                        for k in range(16):
                            nc.tensor.matmul(yp[:], ot[:, k, :],
                                             wo_sb[:, k, ts(on, 512)],
                                             start=(k == 0), stop=(k == 15))
                        ysb = wp.tile([128, 512], f32, tag="ysb", bufs=2)
                        nc.scalar.copy(ysb[:], yp[:])
                        nc.gpsimd.dma_start(
                            y.ap()[b, ts(tc_, 128), ts(on, 512)], ysb[:])

            # ================= pipeline over batches ========================
            for b in range(B):
                qb = kvp.tile([128, HPC, T], bf16, tag="qb", name="qb")
                kb = kvp.tile([128, T], bf16, tag="kb", name="kb")
                vb = kvp.tile([128, NCH, D], bf16, tag="vb", name="vb")
                with nc.named_scope(f"proj{b}", notify=True):
                    proj_batch(b, qb, kb, vb)
                with nc.named_scope(f"attn{b}", notify=True):
                    attn_batch(b, qb, kb, vb)
                if b == 0:
                    load_wo()
                if b >= 1:
                    with nc.named_scope(f"cproj{b - 1}", notify=True):
                        cproj_batch(b - 1)
                emit_a2a(b)
            with nc.named_scope("cproj3", notify=True):
                cproj_batch(B - 1)

    nc.compile()
    return nc


def _get_program():
    global _BUILT
    if _BUILT is None:
        _BUILT = _build_program()
    return _BUILT


def _host_inputs(x, Wq, Wk, Wv, Wo):
    """Per-core input maps (host-side sharding + bf16 layout marshaling)."""
    import ml_dtypes
    bf = ml_dtypes.bfloat16

    x = np.asarray(x, dtype=np.float32)
    Wq = np.asarray(Wq, dtype=np.float32)
    Wk = np.asarray(Wk, dtype=np.float32)
    Wv = np.asarray(Wv, dtype=np.float32)
    Wo = np.asarray(Wo, dtype=np.float32)

    xT = np.ascontiguousarray(x.reshape(BT, C).T.astype(bf))
    woT = np.ascontiguousarray(Wo.T.astype(bf))

    # RoPE tables in (d, t) layout; q tables carry the 1/sqrt(D) scale.
    inv_freq = 1.0 / (ROPE_BASE ** (np.arange(0, D, 2, dtype=np.float32) / D))
    t_ar = np.arange(T, dtype=np.float32)
    freqs = t_ar[:, None] * inv_freq[None, :]          # (T, D/2)
    emb = np.concatenate([freqs, freqs], axis=-1)      # (T, D)
    cos = np.cos(emb).astype(np.float32).T             # (D, T)
    sin = np.sin(emb).astype(np.float32).T
    sgn = np.where(np.arange(D) < D // 2, -1.0, 1.0).astype(np.float32)
    qs = np.float32(1.0 / np.sqrt(D))
    rope_t = np.stack([cos * qs, sin * qs, cos, sin]).astype(bf)  # [4, D, T]
    rope_t = np.ascontiguousarray(rope_t)

    # rotate-half permutation: rot[m] = sgn[m] * q[(m+64) % 128]
    pm = np.zeros((D, D), dtype=np.float32)
    for m in range(D):
        pm[(m + D // 2) % D, m] = sgn[m]
    pm = np.ascontiguousarray(pm.astype(bf))

    # causal band masks for diagonal chunks, S^T layout (s part, t free):
    # cmask[i, m, j] = 0 if j >= i + 128*m else NEG
    i_idx = np.arange(128)[:, None, None]
    m_idx = np.arange(4)[None, :, None]
    j_idx = np.arange(512)[None, None, :]
    cm = np.where(j_idx >= i_idx + 128 * m_idx, 0.0, NEG).astype(np.float32)
    cm = np.ascontiguousarray(cm.astype(bf))

    ones_col = np.ones((128, 1), dtype=bf)
    ident_np = np.eye(128, dtype=np.float32)
    identb_np = np.eye(128, dtype=np.float32).astype(bf)

    in_maps = []
    for c in range(N_CORES):
        g = c // 2
        wq_c = Wq[c * HPC * D:(c + 1) * HPC * D, :].T  # [C, 256]
        wk_c = Wk[g * D:(g + 1) * D, :].T              # [C, 128]
        wv_c = Wv[g * D:(g + 1) * D, :].T              # [C, 128]
        wqkv_c = np.concatenate([wq_c, wk_c, wv_c], axis=1)  # [C, 512]
        in_maps.append({
            "xT": xT,
            "wqkv": np.ascontiguousarray(wqkv_c.astype(bf)),
            "wo": woT,
            "rope": rope_t,
            "perm": pm, "cmask": cm,
            "ones_col": ones_col, "ident": ident_np, "identb": identb_np,
        })
    return in_maps


def kernel(x, attention_mask, Wq, Wk, Wv, Wo):
    """Full inputs in, full output out. attention_mask is all-ones for this
    problem (padding contribution is zero), so only the causal mask applies."""
    global LAST_EXEC_NS
    from concourse.bass_utils import run_bass_kernel_spmd

    nc = _get_program()
    in_maps = _host_inputs(x, Wq, Wk, Wv, Wo)
    res = run_bass_kernel_spmd(nc, in_maps, list(range(N_CORES)), trace=TRACE)
    LAST_EXEC_NS = res.exec_time_ns
    out = np.empty((B, T, C), dtype=np.float32)
    for c in range(N_CORES):
        yc = np.asarray(res.results[c]["y"], dtype=np.float32)  # [B, TPC, C]
        out[:, c * TPC:(c + 1) * TPC, :] = yc
    return out


if __name__ == "__main__":
    _get_program()
    print("program built + compiled OK")


# revision 25
# speedup vs baseline: 1.3104x; 1.3104x over previous
"""Bass/Tile Trainium2 kernel for nn_CausalSelfAttention (B=4, T=2048, C=2048,
H=16 Q-heads, 4 KV-heads, RoPE, causal, fp32) distributed over 8 NeuronCores.

Sharding: tensor-parallel by head. Core c owns Q-heads {2c, 2c+1} and KV-head
c//2 (whole GQA groups). After attention on batch b, a per-batch AllToAll
redistributes the per-head outputs so every core computes the c_proj for a
256-token slice of each batch against the full Wo.

v3 design notes:
  - All stored tensors bf16 (fp32 PSUM accumulation). Besides halving DMA
    and enabling FWL, bf16 lifts the power throttle that pins fp32r matmul
    streams at k=13/16 (~1.95GHz): measured bf16 runs reach k=8/8 (2.4GHz).
  - q/k/v live in per-batch SBUF tiles (no DRAM round-trip), 2-batch
    pipeline via pool rotation.
  - exp runs on pairs of key chunks ([128,2,512], 573ns/chunk) so the ACT
    engine keeps pace with the PE's ~500ns/chunk score+pv stream.
  - Softmax denominator: bf16 DVE running sum of exp chunks, then 4 tiny
    matmuls put the per-query sums on 128 partitions ([128,4]), making the
    reciprocal a ~200ns DVE op. The 1/dn broadcast is a 2-DMA round-trip
    deferred two tiles so no engine FIFO blocks on its latency.
  - Emission order per batch: proj(b) | attn(b) | cproj(b-1) | a2a(b), so
    collectives always have a full batch of compute to hide behind and the
    input DMA queue (sync) never has a collective-dependent load at head.
  - Input DMAs (x, weights, a2a-out reads) on the sync queue; output DMAs
    (a2a-in, y, reciprocal round-trip) + collectives on the gpsimd queue.
  - PSUM: mm[2x2KB] rotation (proj/cproj/transposes/dnt) + sp2[2x4KB]
    (score pairs) + op[2x2KB] (PV accumulators) = exactly 8 banks.
"""

import numpy as np

B, T, C = 4, 2048, 2048
H, KV = 16, 4
D = C // H  # 128
BT = B * T  # 8192
N_CORES = 8
HPC = H // N_CORES  # q heads per core = 2
TPC = T // N_CORES  # tokens per core per batch for c_proj = 256
ROPE_BASE = 10000.0
NEG = -1.0e30

NTB = T // 512  # 4 projection/attention t-tiles per batch
NCH = T // 128  # 16 key chunks per batch

TRACE = False
LAST_EXEC_NS = None

_BUILT = None


def _build_program():
    import concourse.mybir as mybir
    import concourse.tile as tile
    from concourse import bacc
    from concourse.bass import ts

    f32 = mybir.dt.float32
    bf16 = mybir.dt.bfloat16
    Alu = mybir.AluOpType
    Act = mybir.ActivationFunctionType

    nc = bacc.Bacc("TRN2", target_bir_lowering=False, debug=False,
                   num_devices=N_CORES)

    # ---- I/O (all bf16 except the fp32 output) ----
    xT = nc.dram_tensor("xT", [C, BT], bf16, kind="ExternalInput")
    # packed [wq(2 heads) | wk | wv] -> [C, 512]
    wqkv = nc.dram_tensor("wqkv", [C, 4 * D], bf16, kind="ExternalInput")
    wo = nc.dram_tensor("wo", [C, C], bf16, kind="ExternalInput")
    # packed rope tables [4, D, T]: cosq, sinq, cosk, sink (q tables carry
    # the 1/sqrt(D) scale)
    rope = nc.dram_tensor("rope", [4, D, T], bf16, kind="ExternalInput")
    perm = nc.dram_tensor("perm", [D, D], bf16, kind="ExternalInput")
    cmask = nc.dram_tensor("cmask", [128, 4, 512], bf16, kind="ExternalInput")
    ones_col = nc.dram_tensor("ones_col", [128, 1], bf16, kind="ExternalInput")
    ones_row = nc.dram_tensor("ones_row", [1, 128], bf16, kind="ExternalInput")
    ident = nc.dram_tensor("ident", [128, 128], f32, kind="ExternalInput")
    identb = nc.dram_tensor("identb", [128, 128], bf16, kind="ExternalInput")
    y = nc.dram_tensor("y", [B, TPC, C], f32, kind="ExternalOutput")

    with tile.TileContext(nc) as tc:
        with (
            tc.tile_pool(name="const", bufs=1) as cp,
            tc.tile_pool(name="qkv", bufs=2) as kvp,
            tc.tile_pool(name="x", bufs=2) as xp,
            tc.tile_pool(name="work", bufs=2) as wp,
            tc.tile_pool(name="dram", bufs=1, space="DRAM") as dp,
            tc.tile_pool(name="rcp", bufs=4, space="DRAM") as rcp,
            tc.tile_pool(name="psum", bufs=2, space="PSUM") as pp_,
        ):
            xT_r = xT.ap().rearrange("(ko p) t -> p ko t", p=128)

            # ---- startup: first x tile, packed weights, rope tables ----
            xts = {}

            def load_xt(gt):  # gt = global tile index 0..15
                if gt < B * NTB and gt not in xts:
                    xt = xp.tile([128, 16, 512], bf16, tag="xt", name="xt")
                    nc.sync.dma_start(xt[:], xT_r[:, :, ts(gt, 512)])
                    xts[gt] = xt

            wqkv_r = wqkv.ap().rearrange("(ko p) m -> p ko m", p=128)
            wqkv_sb = cp.tile([128, 16, 4 * D], bf16, name="wqkv_sb")
            nc.sync.dma_start(wqkv_sb[:], wqkv_r)
            load_xt(0)
            rope_sb = cp.tile([D, 4, T], bf16, name="rope_sb")
            nc.sync.dma_start(rope_sb[:], rope.ap().rearrange("f d t -> d f t"))
            load_xt(1)
            perm_sb = cp.tile([D, D], bf16)
            nc.sync.dma_start(perm_sb[:], perm.ap())
            cmask_sb = cp.tile([128, 4, 512], bf16)
            nc.sync.dma_start(cmask_sb[:], cmask.ap())
            onec_sb = cp.tile([128, 1], bf16)
            nc.sync.dma_start(onec_sb[:], ones_col.ap())
            oner_sb = cp.tile([1, 128], bf16)
            nc.sync.dma_start(oner_sb[:], ones_row.ap())
            ident_sb = cp.tile([128, 128], f32)
            nc.sync.dma_start(ident_sb[:], ident.ap())
            identb_sb = cp.tile([128, 128], bf16)
            nc.sync.dma_start(identb_sb[:], identb.ap())

            # full Wo resident in SBUF (bf16, 64KB/partition), loaded in
            # quarters during batch 0 (first needed at cproj(0))
            wo_r = wo.ap().rearrange("(ko p) n -> p ko n", p=128)
            wo_sb = cp.tile([128, 16, C], bf16, name="wo_sb")

            def load_wo():
                for q in range(4):
                    nc.sync.dma_start(wo_sb[:, ts(q, 4), :],
                                      wo_r[:, ts(q, 4), :])

            # DRAM collective buffers, one pair per batch
            a2a_in = [dp.tile([N_CORES, HPC, D, TPC], bf16, name=f"a2a_in{b}")
                      for b in range(B)]
            a2a_out = [dp.tile([N_CORES, HPC, D, TPC], bf16, name=f"a2a_out{b}")
                       for b in range(B)]

            # deferred tail queues: stage 1 (reciprocal + broadcast DMA
            # round-trip) runs one attention tile late; stage 2 (normalize
            # multiply + a2a staging) two tiles late, once the broadcast has
            # landed, so no engine FIFO head-blocks on DMA latency.
            pend1, pend2 = [], []

            def flush_pending():
                while pend2:
                    pend2.pop(0)()
                while pend1:
                    pend2.append(pend1.pop(0)())
                while pend2:
                    pend2.pop(0)()

            def step_pending():
                while pend2:
                    pend2.pop(0)()
                while pend1:
                    pend2.append(pend1.pop(0)())

            # ================= phase 1: projections + RoPE (batch b) ========
            def proj_batch(b, qb, kb, vb):
                for tt in range(NTB):
                    gt = b * NTB + tt
                    xt = xts.pop(gt)
                    pos = tt * 512

                    cos_t = [rope_sb[:, 0, pos:pos + 512],
                             rope_sb[:, 0, pos:pos + 512],
                             rope_sb[:, 2, pos:pos + 512]]
                    sin_t = [rope_sb[:, 1, pos:pos + 512],
                             rope_sb[:, 1, pos:pos + 512],
                             rope_sb[:, 3, pos:pos + 512]]

                    pps, evs, t1s = [], [], []
                    for gi in range(4):
                        pqp = pp_.tile([128, 512], f32, tag="mm", bufs=2)
                        for k in range(16):
                            nc.tensor.matmul(pqp[:],
                                             wqkv_sb[:, k, ts(gi, 128)],
                                             xt[:, k, :],
                                             start=(k == 0), stop=(k == 15))
                        if gi < 3:
                            ev = wp.tile([128, 512], bf16, tag="ev", bufs=3)
                            nc.scalar.copy(ev[:], pqp[:])
                            # t1 emitted now so the "mm" slot frees early
                            t1 = wp.tile([128, 512], bf16, tag="t1", bufs=3)
                            nc.vector.tensor_tensor(t1[:], pqp[:], cos_t[gi],
                                                    op=Alu.mult)
                            t1s.append(t1)
                        else:
                            ev = wp.tile([128, 512], f32, tag="ev3", bufs=1)
                            nc.scalar.copy(ev[:], pqp[:])
                        evs.append(ev)

                    # rotate-half perm matmuls (t2 right behind each, so the
                    # mm-slot reader is already queued when the slot recycles)
                    t2s = []
                    for gi in range(3):
                        rp = pp_.tile([128, 512], f32, tag="mm", bufs=2)
                        nc.tensor.matmul(rp[:], perm_sb[:], evs[gi][:],
                                         start=True, stop=True)
                        t2 = wp.tile([128, 512], bf16, tag="t2", bufs=3)
                        nc.vector.tensor_tensor(t2[:], rp[:], sin_t[gi],
                                                op=Alu.mult)
                        t2s.append(t2)
                    # V transposes (fp32 to share the mm tag)
                    tp = pp_.tile([128, 512], f32, tag="mm", bufs=2)
                    for i in range(4):
                        nc.tensor.transpose(tp[:, ts(i, 128)],
                                            evs[3][:, ts(i, 128)], ident_sb[:])

                    load_xt(gt + 2)

                    # rope combine -> SBUF q/k tiles (bf16)
                    dsts = [qb[:, 0, pos:pos + 512], qb[:, 1, pos:pos + 512],
                            kb[:, pos:pos + 512]]
                    for gi in range(3):
                        nc.vector.tensor_tensor(dsts[gi], t1s[gi][:],
                                                t2s[gi][:], op=Alu.add)
                    for i in range(4):
                        nc.scalar.copy(vb[:, 4 * tt + i, :], tp[:, ts(i, 128)])

            # ================= phase 2: attention (batch b) =================
            def attn_batch(b, qb, kb, vb):
                for h in range(HPC):
                    for tt in range(NTB):
                        step_pending()
                        nch = 4 * (tt + 1)
                        npr = nch // 2
                        qt = qb[:, h, ts(tt, 512)]
                        op = pp_.tile([D, 512], f32, tag="op", bufs=2)
                        pts = []
                        acc = None

                        def emit_scores(j):
                            sp = pp_.tile([128, 2, 512], f32, tag="sp2",
                                          bufs=2)
                            for hf in range(2):
                                si = 2 * j + hf
                                diag = si >= 4 * tt
                                nc.tensor.matmul(sp[:, hf, :],
                                                 kb[:, ts(si, 128)], qt,
                                                 start=True, stop=not diag)
                                if diag:
                                    nc.tensor.matmul(
                                        sp[:, hf, :], identb_sb[:],
                                        cmask_sb[:, si - 4 * tt, :],
                                        start=False, stop=True)
                            pt = wp.tile([128, 2, 512], bf16, tag="pt",
                                         bufs=3)
                            nc.scalar.activation(pt[:], sp[:], Act.Exp)
                            pts.append(pt)

                        def emit_pv(j):
                            nonlocal acc
                            pt = pts[j]
                            for hf in range(2):
                                si = 2 * j + hf
                                nc.tensor.matmul(op[:], vb[:, si, :],
                                                 pt[:, hf, :],
                                                 start=(si == 0),
                                                 stop=(si == nch - 1))
                            if j == 0:
                                a = wp.tile([128, 512], bf16, tag="acc",
                                            bufs=2)
                                nc.vector.tensor_tensor(a[:], pt[:, 0, :],
                                                        pt[:, 1, :],
                                                        op=Alu.add)
                            else:
                                a = wp.tile([128, 512], bf16, tag="acc",
                                            bufs=2)
                                nc.vector.tensor_tensor(a[:], acc[:],
                                                        pt[:, 0, :],
                                                        op=Alu.add)
                                nc.vector.tensor_tensor(a[:], a[:],
                                                        pt[:, 1, :],
                                                        op=Alu.add)
                            acc = a

                        emit_scores(0)
                        for j in range(1, npr):
                            emit_scores(j)
                            emit_pv(j - 1)
                        emit_pv(npr - 1)

                        # denominator dn[1,512] = ones.T @ acc (one matmul)
                        dn = pp_.tile([128, 512], f32, tag="mm", bufs=2)
                        nc.tensor.matmul(dn[0:1, :], onec_sb[:], acc[:],
                                         start=True, stop=True)

                        def tail1(b=b, h=h, tt=tt, op=op, dn=dn):
                            # evict dn to SBUF so the PE can outer-product it
                            dnb = wp.tile([1, 512], bf16, tag="dnb", bufs=2)
                            nc.scalar.copy(dnb[:], dn[0:1, :])

                            def tail2(b=b, h=h, tt=tt, op=op, dnb=dnb):
                                # broadcast dn along partitions on the PE,
                                # then 1/dn and the normalize on the DVE —
                                # no DMA round-trip anywhere in this chain
                                bcs = pp_.tile([128, 512], f32, tag="mm",
                                               bufs=2)
                                nc.tensor.matmul(bcs[:], oner_sb[:], dnb[:],
                                                 start=True, stop=True)
                                rcb = wp.tile([128, 512], f32, tag="rcb",
                                              bufs=2)
                                nc.vector.reciprocal_approx_fast(rcb[:],
                                                                 bcs[:])
                                osb = wp.tile([D, 512], bf16, tag="osb",
                                              bufs=2)
                                nc.vector.tensor_tensor(osb[:], op[:], rcb[:],
                                                        op=Alu.mult)
                                # two 256-token shards of the a2a input
                                for half in range(2):
                                    j = 2 * tt + half
                                    nc.gpsimd.dma_start(
                                        a2a_in[b][j, h, :, :],
                                        osb[:, ts(half, 256)])

                            return tail2

                        pend1.append(tail1)

            def emit_a2a(b):
                flush_pending()
                nc.gpsimd.collective_compute(
                    "AllToAll", mybir.AluOpType.bypass,
                    replica_groups=[list(range(N_CORES))],
                    ins=[a2a_in[b].opt()], outs=[a2a_out[b].opt()])

            # ================= phase 3: c_proj (batch b) ====================
            def cproj_batch(b):
                a2a_r = a2a_out[b].rearrange("i h d t -> d (i h) t")
                for tc_ in range(TPC // 128):
                    ot = wp.tile([128, 16, 128], bf16, tag="ot", bufs=2)
                    nc.sync.dma_start(ot[:], a2a_r[:, :, ts(tc_, 128)])
                    for on in range(4):
                        yp = pp_.tile([128, 512], f32, tag="mm", bufs=2)
                        for k in range(16):
                            nc.tensor.matmul(yp[:], ot[:, k, :],
                                             wo_sb[:, k, ts(on, 512)],
                                             start=(k == 0), stop=(k == 15))
                        ysb = wp.tile([128, 512], f32, tag="ysb", bufs=2)
                        nc.scalar.copy(ysb[:], yp[:])
                        nc.gpsimd.dma_start(
                            y.ap()[b, ts(tc_, 128), ts(on, 512)], ysb[:])

            # ================= pipeline over batches ========================
            for b in range(B):
                qb = kvp.tile([128, HPC, T], bf16, tag="qb", name="qb")
                kb = kvp.tile([128, T], bf16, tag="kb", name="kb")
                vb = kvp.tile([128, NCH, D], bf16, tag="vb", name="vb")
                with nc.named_scope(f"proj{b}", notify=True):
                    proj_batch(b, qb, kb, vb)
                with nc.named_scope(f"attn{b}", notify=True):
                    attn_batch(b, qb, kb, vb)
                if b == 0:
                    load_wo()
                if b >= 1:
                    with nc.named_scope(f"cproj{b - 1}", notify=True):
                        cproj_batch(b - 1)
                emit_a2a(b)
            with nc.named_scope("cproj3", notify=True):
                cproj_batch(B - 1)

    nc.compile()
    return nc


def _get_program():
    global _BUILT
    if _BUILT is None:
        _BUILT = _build_program()
    return _BUILT


def _host_inputs(x, Wq, Wk, Wv, Wo):
    """Per-core input maps (host-side sharding + bf16 layout marshaling)."""
    import ml_dtypes
    bf = ml_dtypes.bfloat16

    x = np.asarray(x, dtype=np.float32)
    Wq = np.asarray(Wq, dtype=np.float32)
    Wk = np.asarray(Wk, dtype=np.float32)
    Wv = np.asarray(Wv, dtype=np.float32)
    Wo = np.asarray(Wo, dtype=np.float32)

    xT = np.ascontiguousarray(x.reshape(BT, C).T.astype(bf))
    woT = np.ascontiguousarray(Wo.T.astype(bf))

    # RoPE tables in (d, t) layout; q tables carry the 1/sqrt(D) scale.
    inv_freq = 1.0 / (ROPE_BASE ** (np.arange(0, D, 2, dtype=np.float32) / D))
    t_ar = np.arange(T, dtype=np.float32)
    freqs = t_ar[:, None] * inv_freq[None, :]          # (T, D/2)
    emb = np.concatenate([freqs, freqs], axis=-1)      # (T, D)
    cos = np.cos(emb).astype(np.float32).T             # (D, T)
    sin = np.sin(emb).astype(np.float32).T
    sgn = np.where(np.arange(D) < D // 2, -1.0, 1.0).astype(np.float32)
    qs = np.float32(1.0 / np.sqrt(D))
    rope_t = np.stack([cos * qs, sin * qs, cos, sin]).astype(bf)  # [4, D, T]
    rope_t = np.ascontiguousarray(rope_t)

    # rotate-half permutation: rot[m] = sgn[m] * q[(m+64) % 128]
    pm = np.zeros((D, D), dtype=np.float32)
    for m in range(D):
        pm[(m + D // 2) % D, m] = sgn[m]
    pm = np.ascontiguousarray(pm.astype(bf))

    # causal band masks for diagonal chunks, S^T layout (s part, t free):
    # cmask[i, m, j] = 0 if j >= i + 128*m else NEG
    i_idx = np.arange(128)[:, None, None]
    m_idx = np.arange(4)[None, :, None]
    j_idx = np.arange(512)[None, None, :]
    cm = np.where(j_idx >= i_idx + 128 * m_idx, 0.0, NEG).astype(np.float32)
    cm = np.ascontiguousarray(cm.astype(bf))

    ones_col = np.ones((128, 1), dtype=bf)
    ones_row = np.ones((1, 128), dtype=bf)
    ident_np = np.eye(128, dtype=np.float32)
    identb_np = np.eye(128, dtype=np.float32).astype(bf)

    in_maps = []
    for c in range(N_CORES):
        g = c // 2
        wq_c = Wq[c * HPC * D:(c + 1) * HPC * D, :].T  # [C, 256]
        wk_c = Wk[g * D:(g + 1) * D, :].T              # [C, 128]
        wv_c = Wv[g * D:(g + 1) * D, :].T              # [C, 128]
        wqkv_c = np.concatenate([wq_c, wk_c, wv_c], axis=1)  # [C, 512]
        in_maps.append({
            "xT": xT,
            "wqkv": np.ascontiguousarray(wqkv_c.astype(bf)),
            "wo": woT,
            "rope": rope_t,
            "perm": pm, "cmask": cm,
            "ones_col": ones_col, "ones_row": ones_row,
            "ident": ident_np, "identb": identb_np,
        })
    return in_maps


def kernel(x, attention_mask, Wq, Wk, Wv, Wo):
    """Full inputs in, full output out. attention_mask is all-ones for this
    problem (padding contribution is zero), so only the causal mask applies."""
    global LAST_EXEC_NS
    from concourse.bass_utils import run_bass_kernel_spmd

    nc = _get_program()
    in_maps = _host_inputs(x, Wq, Wk, Wv, Wo)
    res = run_bass_kernel_spmd(nc, in_maps, list(range(N_CORES)), trace=TRACE)
    LAST_EXEC_NS = res.exec_time_ns
    out = np.empty((B, T, C), dtype=np.float32)
    for c in range(N_CORES):
        yc = np.asarray(res.results[c]["y"], dtype=np.float32)  # [B, TPC, C]
        out[:, c * TPC:(c + 1) * TPC, :] = yc
    return out


if __name__ == "__main__":
    _get_program()
    print("program built + compiled OK")


# revision 28
# speedup vs baseline: 1.5255x; 1.1642x over previous
"""Bass/Tile Trainium2 kernel for nn_CausalSelfAttention (B=4, T=2048, C=2048,
H=16 Q-heads, 4 KV-heads, RoPE, causal, fp32) distributed over 8 NeuronCores.

Sharding: tensor-parallel by head. Core c owns Q-heads {2c, 2c+1} and KV-head
c//2 (whole GQA groups). After attention on batch b, a per-batch AllToAll
redistributes the per-head outputs so every core computes the c_proj for a
256-token slice of each batch against the full Wo.

v3 design notes:
  - All stored tensors bf16 (fp32 PSUM accumulation). Besides halving DMA
    and enabling FWL, bf16 lifts the power throttle that pins fp32r matmul
    streams at k=13/16 (~1.95GHz): measured bf16 runs reach k=8/8 (2.4GHz).
  - q/k/v live in per-batch SBUF tiles (no DRAM round-trip), 2-batch
    pipeline via pool rotation.
  - exp runs on pairs of key chunks ([128,2,512], 573ns/chunk) so the ACT
    engine keeps pace with the PE's ~500ns/chunk score+pv stream.
  - Softmax denominator: bf16 DVE running sum of exp chunks, then 4 tiny
    matmuls put the per-query sums on 128 partitions ([128,4]), making the
    reciprocal a ~200ns DVE op. The 1/dn broadcast is a 2-DMA round-trip
    deferred two tiles so no engine FIFO blocks on its latency.
  - Emission order per batch: proj(b) | attn(b) | cproj(b-1) | a2a(b), so
    collectives always have a full batch of compute to hide behind and the
    input DMA queue (sync) never has a collective-dependent load at head.
  - Input DMAs (x, weights, a2a-out reads) on the sync queue; output DMAs
    (a2a-in, y, reciprocal round-trip) + collectives on the gpsimd queue.
  - PSUM: mm[2x2KB] rotation (proj/cproj/transposes/dnt) + sp2[2x4KB]
    (score pairs) + op[2x2KB] (PV accumulators) = exactly 8 banks.
"""

import numpy as np

B, T, C = 4, 2048, 2048
H, KV = 16, 4
D = C // H  # 128
BT = B * T  # 8192
N_CORES = 8
HPC = H // N_CORES  # q heads per core = 2
TPC = T // N_CORES  # tokens per core per batch for c_proj = 256
ROPE_BASE = 10000.0
NEG = -1.0e30

NTB = T // 512  # 4 projection/attention t-tiles per batch
NCH = T // 128  # 16 key chunks per batch

TRACE = False
LAST_EXEC_NS = None

_BUILT = None


def _build_program():
    import concourse.mybir as mybir
    import concourse.tile as tile
    from concourse import bacc
    from concourse.bass import ts

    f32 = mybir.dt.float32
    bf16 = mybir.dt.bfloat16
    Alu = mybir.AluOpType
    Act = mybir.ActivationFunctionType

    nc = bacc.Bacc("TRN2", target_bir_lowering=False, debug=False,
                   num_devices=N_CORES)

    # ---- I/O (all bf16 except the fp32 output) ----
    xT = nc.dram_tensor("xT", [C, BT], bf16, kind="ExternalInput")
    # packed [wq(2 heads) | wk | wv] -> [C, 512]
    wqkv = nc.dram_tensor("wqkv", [C, 4 * D], bf16, kind="ExternalInput")
    wo = nc.dram_tensor("wo", [C, C], bf16, kind="ExternalInput")
    # packed rope tables [4, D, T]: cosq, sinq, cosk, sink (q tables carry
    # the 1/sqrt(D) scale)
    rope = nc.dram_tensor("rope", [4, D, T], bf16, kind="ExternalInput")
    perm = nc.dram_tensor("perm", [D, D], bf16, kind="ExternalInput")
    cmask = nc.dram_tensor("cmask", [128, 4, 512], bf16, kind="ExternalInput")
    ones_col = nc.dram_tensor("ones_col", [128, 1], bf16, kind="ExternalInput")
    ones_row = nc.dram_tensor("ones_row", [1, 128], bf16, kind="ExternalInput")
    ident = nc.dram_tensor("ident", [128, 128], f32, kind="ExternalInput")
    identb = nc.dram_tensor("identb", [128, 128], bf16, kind="ExternalInput")
    y = nc.dram_tensor("y", [B, TPC, C], f32, kind="ExternalOutput")

    with tile.TileContext(nc) as tc:
        with (
            tc.tile_pool(name="const", bufs=1) as cp,
            tc.tile_pool(name="qkv", bufs=2) as kvp,
            tc.tile_pool(name="x", bufs=2) as xp,
            tc.tile_pool(name="work", bufs=2) as wp,
            tc.tile_pool(name="dram", bufs=1, space="DRAM") as dp,
            tc.tile_pool(name="rcp", bufs=4, space="DRAM") as rcp,
            tc.tile_pool(name="psum", bufs=2, space="PSUM") as pp_,
        ):
            xT_r = xT.ap().rearrange("(ko p) t -> p ko t", p=128)

            # ---- startup: first x tile, packed weights, rope tables ----
            xts = {}

            def load_xt(gt):  # gt = global tile index 0..15
                if gt < B * NTB and gt not in xts:
                    xt = xp.tile([128, 16, 512], bf16, tag="xt", name="xt")
                    nc.sync.dma_start(xt[:], xT_r[:, :, ts(gt, 512)])
                    xts[gt] = xt

            wqkv_r = wqkv.ap().rearrange("(ko p) m -> p ko m", p=128)
            wqkv_sb = cp.tile([128, 16, 4 * D], bf16, name="wqkv_sb")
            nc.sync.dma_start(wqkv_sb[:], wqkv_r)
            load_xt(0)
            rope_sb = cp.tile([D, 4, T], bf16, name="rope_sb")
            nc.sync.dma_start(rope_sb[:], rope.ap().rearrange("f d t -> d f t"))
            load_xt(1)
            perm_sb = cp.tile([D, D], bf16)
            nc.sync.dma_start(perm_sb[:], perm.ap())
            cmask_sb = cp.tile([128, 4, 512], bf16)
            nc.sync.dma_start(cmask_sb[:], cmask.ap())
            onec_sb = cp.tile([128, 1], bf16)
            nc.sync.dma_start(onec_sb[:], ones_col.ap())
            oner_sb = cp.tile([1, 128], bf16)
            nc.sync.dma_start(oner_sb[:], ones_row.ap())
            ident_sb = cp.tile([128, 128], f32)
            nc.sync.dma_start(ident_sb[:], ident.ap())
            identb_sb = cp.tile([128, 128], bf16)
            nc.sync.dma_start(identb_sb[:], identb.ap())

            # full Wo resident in SBUF (bf16, 64KB/partition), loaded in
            # quarters during batch 0 (first needed at cproj(0))
            wo_r = wo.ap().rearrange("(ko p) n -> p ko n", p=128)
            wo_sb = cp.tile([128, 16, C], bf16, name="wo_sb")

            def load_wo():
                for q in range(4):
                    nc.sync.dma_start(wo_sb[:, ts(q, 4), :],
                                      wo_r[:, ts(q, 4), :])

            # DRAM collective buffers, one pair per batch
            a2a_in = [dp.tile([N_CORES, HPC, D, TPC], bf16, name=f"a2a_in{b}")
                      for b in range(B)]
            a2a_out = [dp.tile([N_CORES, HPC, D, TPC], bf16, name=f"a2a_out{b}")
                       for b in range(B)]

            # tiny warmup AllToAll fired immediately: the first collective
            # call pays ~100us of ncfw/ENCD setup; let it overlap proj(0)
            warm_in = dp.tile([N_CORES, 128], bf16, name="warm_in")
            warm_out = dp.tile([N_CORES, 128], bf16, name="warm_out")
            nc.gpsimd.collective_compute(
                "AllToAll", mybir.AluOpType.bypass,
                replica_groups=[list(range(N_CORES))],
                ins=[warm_in.opt()], outs=[warm_out.opt()])

            # deferred tail queues: stage 1 (reciprocal + broadcast DMA
            # round-trip) runs one attention tile late; stage 2 (normalize
            # multiply + a2a staging) two tiles late, once the broadcast has
            # landed, so no engine FIFO head-blocks on DMA latency.
            pend1, pend2 = [], []

            def flush_pending():
                while pend2:
                    pend2.pop(0)()
                while pend1:
                    pend2.append(pend1.pop(0)())
                while pend2:
                    pend2.pop(0)()

            def step_pending():
                while pend2:
                    pend2.pop(0)()
                while pend1:
                    pend2.append(pend1.pop(0)())

            # ================= phase 1: projections + RoPE (batch b) ========
            def proj_batch(b, qb, kb, vb):
                for tt in range(NTB):
                    gt = b * NTB + tt
                    xt = xts.pop(gt)
                    pos = tt * 512

                    cos_t = [rope_sb[:, 0, pos:pos + 512],
                             rope_sb[:, 0, pos:pos + 512],
                             rope_sb[:, 2, pos:pos + 512]]
                    sin_t = [rope_sb[:, 1, pos:pos + 512],
                             rope_sb[:, 1, pos:pos + 512],
                             rope_sb[:, 3, pos:pos + 512]]

                    pps, evs, t1s = [], [], []
                    for gi in range(4):
                        pqp = pp_.tile([128, 512], f32, tag="mm", bufs=2)
                        for k in range(16):
                            nc.tensor.matmul(pqp[:],
                                             wqkv_sb[:, k, ts(gi, 128)],
                                             xt[:, k, :],
                                             start=(k == 0), stop=(k == 15))
                        if gi < 3:
                            ev = wp.tile([128, 512], bf16, tag="ev", bufs=3)
                            nc.scalar.copy(ev[:], pqp[:])
                            # t1 emitted now so the "mm" slot frees early
                            t1 = wp.tile([128, 512], bf16, tag="t1", bufs=3)
                            nc.vector.tensor_tensor(t1[:], pqp[:], cos_t[gi],
                                                    op=Alu.mult)
                            t1s.append(t1)
                        else:
                            ev = wp.tile([128, 512], f32, tag="ev3", bufs=1)
                            nc.scalar.copy(ev[:], pqp[:])
                        evs.append(ev)

                    # rotate-half perm matmuls (t2 right behind each, so the
                    # mm-slot reader is already queued when the slot recycles)
                    t2s = []
                    for gi in range(3):
                        rp = pp_.tile([128, 512], f32, tag="mm", bufs=2)
                        nc.tensor.matmul(rp[:], perm_sb[:], evs[gi][:],
                                         start=True, stop=True)
                        t2 = wp.tile([128, 512], bf16, tag="t2", bufs=3)
                        nc.vector.tensor_tensor(t2[:], rp[:], sin_t[gi],
                                                op=Alu.mult)
                        t2s.append(t2)
                    # V transposes (fp32 to share the mm tag)
                    tp = pp_.tile([128, 512], f32, tag="mm", bufs=2)
                    for i in range(4):
                        nc.tensor.transpose(tp[:, ts(i, 128)],
                                            evs[3][:, ts(i, 128)], ident_sb[:])

                    load_xt(gt + 2)

                    # rope combine -> SBUF q/k tiles (bf16)
                    dsts = [qb[:, 0, pos:pos + 512], qb[:, 1, pos:pos + 512],
                            kb[:, pos:pos + 512]]
                    for gi in range(3):
                        nc.vector.tensor_tensor(dsts[gi], t1s[gi][:],
                                                t2s[gi][:], op=Alu.add)
                    for i in range(4):
                        nc.scalar.copy(vb[:, 4 * tt + i, :], tp[:, ts(i, 128)])

            # ================= phase 2: attention (batch b) =================
            def attn_batch(b, qb, kb, vb):
                for h in range(HPC):
                    for tt in range(NTB):
                        step_pending()
                        nch = 4 * (tt + 1)
                        npr = nch // 2
                        qt = qb[:, h, ts(tt, 512)]
                        op = pp_.tile([D, 512], f32, tag="op", bufs=2)
                        pts = []
                        acc = None

                        def emit_scores(j):
                            sp = pp_.tile([128, 2, 512], f32, tag="sp2",
                                          bufs=2)
                            for hf in range(2):
                                si = 2 * j + hf
                                diag = si >= 4 * tt
                                nc.tensor.matmul(sp[:, hf, :],
                                                 kb[:, ts(si, 128)], qt,
                                                 start=True, stop=not diag)
                                if diag:
                                    nc.tensor.matmul(
                                        sp[:, hf, :], identb_sb[:],
                                        cmask_sb[:, si - 4 * tt, :],
                                        start=False, stop=True)
                            pt = wp.tile([128, 2, 512], bf16, tag="pt",
                                         bufs=3)
                            nc.scalar.activation(pt[:], sp[:], Act.Exp)
                            pts.append(pt)

                        def emit_pv(j):
                            nonlocal acc
                            pt = pts[j]
                            for hf in range(2):
                                si = 2 * j + hf
                                nc.tensor.matmul(op[:], vb[:, si, :],
                                                 pt[:, hf, :],
                                                 start=(si == 0),
                                                 stop=(si == nch - 1))
                            if j == 0:
                                a = wp.tile([128, 512], bf16, tag="acc",
                                            bufs=2)
                                nc.vector.tensor_tensor(a[:], pt[:, 0, :],
                                                        pt[:, 1, :],
                                                        op=Alu.add)
                            else:
                                a = wp.tile([128, 512], bf16, tag="acc",
                                            bufs=2)
                                nc.vector.tensor_tensor(a[:], acc[:],
                                                        pt[:, 0, :],
                                                        op=Alu.add)
                                nc.vector.tensor_tensor(a[:], a[:],
                                                        pt[:, 1, :],
                                                        op=Alu.add)
                            acc = a

                        emit_scores(0)
                        for j in range(1, npr):
                            emit_scores(j)
                            emit_pv(j - 1)
                        emit_pv(npr - 1)

                        # denominator dn[1,512] = ones.T @ acc (one matmul)
                        dn = pp_.tile([128, 512], f32, tag="mm", bufs=2)
                        nc.tensor.matmul(dn[0:1, :], onec_sb[:], acc[:],
                                         start=True, stop=True)

                        def tail1(b=b, h=h, tt=tt, op=op, dn=dn):
                            # evict dn to SBUF so the PE can outer-product it
                            dnb = wp.tile([1, 512], bf16, tag="dnb", bufs=2)
                            nc.scalar.copy(dnb[:], dn[0:1, :])

                            def tail2(b=b, h=h, tt=tt, op=op, dnb=dnb):
                                # broadcast dn along partitions on the PE,
                                # then 1/dn and the normalize on the DVE —
                                # no DMA round-trip anywhere in this chain
                                bcs = pp_.tile([128, 512], f32, tag="mm",
                                               bufs=2)
                                nc.tensor.matmul(bcs[:], oner_sb[:], dnb[:],
                                                 start=True, stop=True)
                                rcb = wp.tile([128, 512], f32, tag="rcb",
                                              bufs=2)
                                nc.vector.reciprocal_approx_fast(rcb[:],
                                                                 bcs[:])
                                osb = wp.tile([D, 512], bf16, tag="osb",
                                              bufs=2)
                                nc.vector.tensor_tensor(osb[:], op[:], rcb[:],
                                                        op=Alu.mult)
                                # two 256-token shards of the a2a input
                                for half in range(2):
                                    j = 2 * tt + half
                                    nc.gpsimd.dma_start(
                                        a2a_in[b][j, h, :, :],
                                        osb[:, ts(half, 256)])

                            return tail2

                        pend1.append(tail1)

            def emit_a2a(b):
                flush_pending()
                nc.gpsimd.collective_compute(
                    "AllToAll", mybir.AluOpType.bypass,
                    replica_groups=[list(range(N_CORES))],
                    ins=[a2a_in[b].opt()], outs=[a2a_out[b].opt()])

            # ================= phase 3: c_proj (batch b) ====================
            def cproj_batch(b):
                a2a_r = a2a_out[b].rearrange("i h d t -> d (i h) t")
                for tc_ in range(TPC // 128):
                    ot = wp.tile([128, 16, 128], bf16, tag="ot", bufs=2)
                    nc.sync.dma_start(ot[:], a2a_r[:, :, ts(tc_, 128)])
                    for on in range(4):
                        yp = pp_.tile([128, 512], f32, tag="mm", bufs=2)
                        for k in range(16):
                            nc.tensor.matmul(yp[:], ot[:, k, :],
                                             wo_sb[:, k, ts(on, 512)],
                                             start=(k == 0), stop=(k == 15))
                        ysb = wp.tile([128, 512], f32, tag="ysb", bufs=2)
                        nc.scalar.copy(ysb[:], yp[:])
                        nc.gpsimd.dma_start(
                            y.ap()[b, ts(tc_, 128), ts(on, 512)], ysb[:])

            # ================= pipeline over batches ========================
            for b in range(B):
                qb = kvp.tile([128, HPC, T], bf16, tag="qb", name="qb")
                kb = kvp.tile([128, T], bf16, tag="kb", name="kb")
                vb = kvp.tile([128, NCH, D], bf16, tag="vb", name="vb")
                with nc.named_scope(f"proj{b}", notify=True):
                    proj_batch(b, qb, kb, vb)
                with nc.named_scope(f"attn{b}", notify=True):
                    attn_batch(b, qb, kb, vb)
                emit_a2a(b)
                if b == 0:
                    load_wo()
                if b >= 1:
                    with nc.named_scope(f"cproj{b - 1}", notify=True):
                        cproj_batch(b - 1)
            with nc.named_scope("cproj3", notify=True):
                cproj_batch(B - 1)

    nc.compile()
    return nc


def _get_program():
    global _BUILT
    if _BUILT is None:
        _BUILT = _build_program()
    return _BUILT


def _host_inputs(x, Wq, Wk, Wv, Wo):
    """Per-core input maps (host-side sharding + bf16 layout marshaling)."""
    import ml_dtypes
    bf = ml_dtypes.bfloat16

    x = np.asarray(x, dtype=np.float32)
    Wq = np.asarray(Wq, dtype=np.float32)
    Wk = np.asarray(Wk, dtype=np.float32)
    Wv = np.asarray(Wv, dtype=np.float32)
    Wo = np.asarray(Wo, dtype=np.float32)

    xT = np.ascontiguousarray(x.reshape(BT, C).T.astype(bf))
    woT = np.ascontiguousarray(Wo.T.astype(bf))

    # RoPE tables in (d, t) layout; q tables carry the 1/sqrt(D) scale.
    inv_freq = 1.0 / (ROPE_BASE ** (np.arange(0, D, 2, dtype=np.float32) / D))
    t_ar = np.arange(T, dtype=np.float32)
    freqs = t_ar[:, None] * inv_freq[None, :]          # (T, D/2)
    emb = np.concatenate([freqs, freqs], axis=-1)      # (T, D)
    cos = np.cos(emb).astype(np.float32).T             # (D, T)
    sin = np.sin(emb).astype(np.float32).T
    sgn = np.where(np.arange(D) < D // 2, -1.0, 1.0).astype(np.float32)
    qs = np.float32(1.0 / np.sqrt(D))
    rope_t = np.stack([cos * qs, sin * qs, cos, sin]).astype(bf)  # [4, D, T]
    rope_t = np.ascontiguousarray(rope_t)

    # rotate-half permutation: rot[m] = sgn[m] * q[(m+64) % 128]
    pm = np.zeros((D, D), dtype=np.float32)
    for m in range(D):
        pm[(m + D // 2) % D, m] = sgn[m]
    pm = np.ascontiguousarray(pm.astype(bf))

    # causal band masks for diagonal chunks, S^T layout (s part, t free):
    # cmask[i, m, j] = 0 if j >= i + 128*m else NEG
    i_idx = np.arange(128)[:, None, None]
    m_idx = np.arange(4)[None, :, None]
    j_idx = np.arange(512)[None, None, :]
    cm = np.where(j_idx >= i_idx + 128 * m_idx, 0.0, NEG).astype(np.float32)
    cm = np.ascontiguousarray(cm.astype(bf))

    ones_col = np.ones((128, 1), dtype=bf)
    ones_row = np.ones((1, 128), dtype=bf)
    ident_np = np.eye(128, dtype=np.float32)
    identb_np = np.eye(128, dtype=np.float32).astype(bf)

    in_maps = []
    for c in range(N_CORES):
        g = c // 2
        wq_c = Wq[c * HPC * D:(c + 1) * HPC * D, :].T  # [C, 256]
        wk_c = Wk[g * D:(g + 1) * D, :].T              # [C, 128]
        wv_c = Wv[g * D:(g + 1) * D, :].T              # [C, 128]
        wqkv_c = np.concatenate([wq_c, wk_c, wv_c], axis=1)  # [C, 512]
        in_maps.append({
            "xT": xT,
            "wqkv": np.ascontiguousarray(wqkv_c.astype(bf)),
            "wo": woT,
            "rope": rope_t,
            "perm": pm, "cmask": cm,
            "ones_col": ones_col, "ones_row": ones_row,
            "ident": ident_np, "identb": identb_np,
        })
    return in_maps


def kernel(x, attention_mask, Wq, Wk, Wv, Wo):
    """Full inputs in, full output out. attention_mask is all-ones for this
    problem (padding contribution is zero), so only the causal mask applies."""
    global LAST_EXEC_NS
    from concourse.bass_utils import run_bass_kernel_spmd

    nc = _get_program()
    in_maps = _host_inputs(x, Wq, Wk, Wv, Wo)
    res = run_bass_kernel_spmd(nc, in_maps, list(range(N_CORES)), trace=TRACE)
    LAST_EXEC_NS = res.exec_time_ns
    out = np.empty((B, T, C), dtype=np.float32)
    for c in range(N_CORES):
        yc = np.asarray(res.results[c]["y"], dtype=np.float32)  # [B, TPC, C]
        out[:, c * TPC:(c + 1) * TPC, :] = yc
    return out


if __name__ == "__main__":
    _get_program()
    print("program built + compiled OK")
